# revision 1
# baseline (speedup 1.0000x reference)
"""CosineSimilarityAttention Trainium2 kernel v2 (8 NeuronCores, SPMD).

Sharding: token-parallel. Core c handles batch (c // 4), query rows
(c % 4)*1024 .. +1024. Each core projects K/V for its whole batch
(replicated within each 4-core batch group) plus Q for its own tokens,
then attention and the output projection for its slice.

v2 vs baseline:
 - all inputs pre-converted to bf16 on host (no on-device casts)
 - 2-pass key sweep: K/V projected per pass half, attention accumulates
   in PSUM with an SBUF spill between passes (halves persistent SBUF)
 - attention inner loop qh-outer so only 2 PSUM accumulators live
 - kf staging via scalar-engine copies/squares (keeps DVE free)
 - no zero-padded qhat: QK uses 64-partition matmuls at base 0/64
"""

import numpy as np
import ml_dtypes

import concourse.bass as bass
import concourse.mybir as mybir
import concourse.tile as tile
from concourse.bass_utils import run_bass_kernel_spmd
from concourse.masks import make_identity

F32 = mybir.dt.float32
BF16 = mybir.dt.bfloat16
AF = mybir.ActivationFunctionType

B = 2
N = 4096          # tokens per batch
D = 768           # model dim
H = 12            # heads
DH = 64           # head dim
INNER = H * DH    # 768
EPS = 1e-8
NQ = 1024         # query tokens per core
NCORES = 8
BLK = 512         # projection token block
PASSES = 2
KBP = N // 128 // PASSES   # 128-token key blocks per pass


def _split_multi_waits(nc):
    """This container's walrus accepts only ONE sync-wait per instruction."""
    n = 0
    for f in nc.m.functions:
        for bb in f.blocks:
            insts = list(bb.instructions)
            out = []
            for inst in insts:
                si = inst.sync_info
                if si is not None and si.on_wait is not None and len(si.on_wait) > 1:
                    waits = list(si.on_wait)
                    for j, w in enumerate(waits[:-1]):
                        ev = mybir.InstEventSemaphore(
                            name=f"{inst.name}-evw{j}",
                            engine=inst.engine,
                            sync_info=mybir.SyncInfo(on_wait=[w], on_update=[]),
                        )
                        out.append(ev)
                        n += 1
                    si.on_wait = [waits[-1]]
                out.append(inst)
            bb.instructions = out
    return n


def _proj_block(nc, pools, wq, sel_bf, ident, eps_t, src, gblk, qcols):
    """Project one 512-token block: returns (xT tile, kfs tile, rq tile).
    qcols: column base in wqkvT (0 for Q, INNER for K)."""
    (stage, xTp, sqp, smallp, pA, pB, pT) = pools
    xst = stage.tile([128, 4, D], BF16, tag="xst")
    nc.sync.dma_start(
        out=xst,
        in_=src[gblk * BLK:(gblk + 1) * BLK, :].rearrange(
            "(t p) d -> p t d", p=128))
    xT = xTp.tile([128, 6, BLK], BF16, tag="xT")
    for dt in range(6):
        tp = pT.tile([128, BLK], BF16, tag="tp")
        for tt in range(4):
            nc.tensor.transpose(
                tp[:, tt * 128:(tt + 1) * 128],
                xst[:, tt, dt * 128:(dt + 1) * 128], ident)
        nc.vector.tensor_copy(xT[:, dt, :], tp)
    kfs = sqp.tile([128, 6, BLK], BF16, tag="kfs")
    sq = pB.tile([128, BLK], F32, tag="sq")
    for dt in range(6):
        kf = pA.tile([128, BLK], F32, tag="kf")
        for ks in range(6):
            nc.tensor.matmul(
                kf, wq[:, ks, qcols + dt * 128:qcols + (dt + 1) * 128],
                xT[:, ks, :], start=(ks == 0), stop=(ks == 5))
        nc.scalar.copy(kfs[:, dt, :], kf)
        ksq = sqp.tile([128, BLK], BF16, tag="ksq")
        nc.scalar.square(ksq, kf)
        nc.tensor.matmul(sq, sel_bf, ksq, start=(dt == 0), stop=(dt == 5))
    nrm = smallp.tile([128, BLK], F32, tag="nrm")
    nc.scalar.activation(nrm, sq, AF.Sqrt)
    u = smallp.tile([128, BLK], F32, tag="u")
    nc.scalar.activation(u, nrm, AF.Sqrt, bias=eps_t[:, :])
    rq = smallp.tile([128, BLK], F32, tag="rq")
    nc.vector.reciprocal(rq, u)
    return xT, kfs, rq


def _build_program(inv_scale):
    nc = bass.Bass()
    xb = nc.declare_dram_parameter("xb", [N, D], BF16, isOutput=False)
    qx = nc.declare_dram_parameter("qx", [NQ, D], BF16, isOutput=False)
    wqkvT = nc.declare_dram_parameter("wqkvT", [D, 3 * INNER], BF16, isOutput=False)
    woT = nc.declare_dram_parameter("woT", [INNER, D], BF16, isOutput=False)
    bout = nc.declare_dram_parameter("bout", [1, D], BF16, isOutput=False)
    selin = nc.declare_dram_parameter("selin", [128, 128], BF16, isOutput=False)
    y = nc.declare_dram_parameter("y", [NQ, D], F32, isOutput=True)

    with tile.TileContext(nc) as tc:
        with tc.tile_pool(name="const", bufs=1) as constp, \
             tc.tile_pool(name="persist", bufs=1) as persist:
            # ---------------- constants + weights ----------------
            ident = constp.tile([128, 128], BF16)
            make_identity(nc, ident)
            sel_bf = constp.tile([128, 128], BF16)
            nc.sync.dma_start(out=sel_bf, in_=selin[:, :])
            b_bf = constp.tile([1, D], BF16)
            ones_row = constp.tile([1, 128], BF16)
            nc.vector.memset(ones_row, 1.0)
            ones_f = constp.tile([1, 64], BF16)
            nc.vector.memset(ones_f, 1.0)
            eps_t = constp.tile([128, 1], F32)
            nc.vector.memset(eps_t, EPS)
            invs = constp.tile([128, 6], F32)
            for dt in range(6):
                nc.vector.memset(invs[0:64, dt:dt + 1], float(inv_scale[2 * dt]))
                nc.vector.memset(invs[64:128, dt:dt + 1],
                                 float(inv_scale[2 * dt + 1]))

            wq = persist.tile([128, 6, 3 * INNER], BF16)
            for dt in range(6):
                nc.sync.dma_start(out=wq[:, dt, :],
                                  in_=wqkvT[dt * 128:(dt + 1) * 128, :])
            wo12 = persist.tile([64, H, D], BF16)

            # ---------------- persistent activations ----------------
            qhat = persist.tile([128, 6, NQ], BF16)
            khat = persist.tile([128, 6, KBP * 128], BF16)
            vhat = persist.tile([128, KBP, H * 65], BF16)
            oh_all = persist.tile([64, H, NQ], BF16)
            ospill = persist.tile([65, 6, 2, 2, 512], BF16)   # [hp, j, qh]

            vones = vhat.rearrange("p t (h c) -> p t h c", c=65)[:, :, :, 64:65]
            nc.vector.memset(vones, 1.0)

            # ---------------- Q projection ----------------
            with tc.tile_pool(name="qstage", bufs=2) as stage, \
                 tc.tile_pool(name="qxT", bufs=2) as xTp, \
                 tc.tile_pool(name="qsq", bufs=1) as sqp, \
                 tc.tile_pool(name="qsmall", bufs=1) as smallp, \
                 tc.tile_pool(name="qpsA", bufs=2, space="PSUM") as pA, \
                 tc.tile_pool(name="qpsB", bufs=1, space="PSUM") as pB, \
                 tc.tile_pool(name="qpsT", bufs=2, space="PSUM") as pT:
                pools = (stage, xTp, sqp, smallp, pA, pB, pT)
                for blk in range(NQ // BLK):
                    xT, kfs, rq = _proj_block(nc, pools, wq, sel_bf, ident,
                                              eps_t, qx, blk, 0)
                    bsl = bass.ts(blk, BLK)
                    for dt in range(6):
                        rqi = smallp.tile([128, BLK], F32, tag="rqi")
                        nc.vector.tensor_scalar_mul(rqi, rq, invs[:, dt:dt + 1])
                        nc.vector.tensor_mul(qhat[:, dt, bsl], kfs[:, dt, :], rqi)

            # ---------------- pass loop: K/V proj + attention ----------------
            for p in range(PASSES):
                with tc.tile_pool(name="kstage", bufs=2) as stage, \
                     tc.tile_pool(name="kxT", bufs=2) as xTp, \
                     tc.tile_pool(name="ksq", bufs=1) as sqp, \
                     tc.tile_pool(name="ksmall", bufs=1) as smallp, \
                     tc.tile_pool(name="kpsA", bufs=2, space="PSUM") as pA, \
                     tc.tile_pool(name="kpsB", bufs=1, space="PSUM") as pB, \
                     tc.tile_pool(name="kpsV", bufs=1, space="PSUM") as pV, \
                     tc.tile_pool(name="kpsT", bufs=2, space="PSUM") as pT:
                    pools = (stage, xTp, sqp, smallp, pA, pB, pT)
                    for blk in range(KBP // 4):
                        gblk = p * (KBP // 4) + blk
                        xT, kfs, rq = _proj_block(nc, pools, wq, sel_bf, ident,
                                                  eps_t, xb, gblk, INNER)
                        bsl = bass.ts(blk, BLK)
                        for dt in range(6):
                            nc.vector.tensor_mul(khat[:, dt, bsl],
                                                 kfs[:, dt, :], rq)
                        # V projection per 128-token tile
                        for tt in range(4):
                            kb = blk * 4 + tt
                            vp = pV.tile([128, INNER], F32, tag="vp")
                            for ks in range(6):
                                nc.tensor.matmul(
                                    vp[:, 0:512],
                                    xT[:, ks, tt * 128:(tt + 1) * 128],
                                    wq[:, ks, 2 * INNER:2 * INNER + 512],
                                    start=(ks == 0), stop=(ks == 5))
                                nc.tensor.matmul(
                                    vp[:, 512:768],
                                    xT[:, ks, tt * 128:(tt + 1) * 128],
                                    wq[:, ks, 2 * INNER + 512:3 * INNER],
                                    start=(ks == 0), stop=(ks == 5))
                            vdst = vhat[:, kb, :].rearrange(
                                "p (h c) -> p h c", c=65)[:, :, 0:64]
                            nc.vector.tensor_copy(
                                vdst,
                                vp[:, 0:768].rearrange("p (h c) -> p h c", c=64))

                # ---- attention for this pass ----
                with tc.tile_pool(name="pS", bufs=2, space="PSUM") as pS, \
                     tc.tile_pool(name="pO", bufs=3, space="PSUM") as pO, \
                     tc.tile_pool(name="pR", bufs=1, space="PSUM") as pR, \
                     tc.tile_pool(name="pt16", bufs=4) as pt16p, \
                     tc.tile_pool(name="posb", bufs=10) as posb, \
                     tc.tile_pool(name="princ", bufs=4) as princ:
                    deferred = []
                    pending = []

                    def emit_tails(keep=0):
                        while len(deferred) > keep:
                            h, qsl2, osum = deferred.pop(0)
                            rinv = princ.tile([1, 512], F32, tag="rinv",
                                              name=f"rinv_{h}_{qsl2}")
                            nc.vector.reciprocal(rinv, osum[64:65, :])
                            rinvr = princ.tile([1, 512], BF16,
                                               tag="rinvr",
                                               name=f"rinvr_{h}_{qsl2}")
                            nc.vector.tensor_copy(rinvr, rinv)
                            rbc = pR.tile([128, 512], F32, tag="rbc",
                                          name=f"rbc_{h}_{qsl2}")
                            nc.tensor.matmul(rbc[0:64, :], ones_f, rinvr,
                                             start=True, stop=True)
                            nc.vector.tensor_mul(
                                oh_all[:, h, bass.ts(qsl2, 512)],
                                osum[0:64, :], rbc[0:64, :])

                    for hp in range(6):
                        for qh in range(2):
                            qsl = bass.ts(qh, 512)
                            ots = [pO.tile([65, 512], F32, tag="pO",
                                           name=f"ot{p}_{hp}_{qh}_{j}")
                                   for j in range(2)]
                            for kb in range(KBP):
                                st = pS.tile([128, 1024], F32, tag="pS",
                                             name=f"st{p}_{hp}_{qh}_{kb}")
                                ksl = bass.ts(kb, 128)
                                nc.tensor.matmul(st[:, 0:512],
                                                 khat[0:64, hp, ksl],
                                                 qhat[0:64, hp, qsl],
                                                 start=True, stop=True)
                                nc.tensor.matmul(st[:, 512:1024],
                                                 khat[64:128, hp, ksl],
                                                 qhat[64:128, hp, qsl],
                                                 start=True, stop=True)
                                pt = pt16p.tile([128, 1024], BF16, tag="pt16",
                                                name=f"pt{p}_{hp}_{qh}_{kb}")
                                nc.scalar.activation(pt, st, AF.Exp)
                                for j in range(2):
                                    h = 2 * hp + j
                                    nc.tensor.matmul(
                                        ots[j],
                                        vhat[:, kb, h * 65:(h + 1) * 65],
                                        pt[:, j * 512:(j + 1) * 512],
                                        start=(kb == 0), stop=(kb == KBP - 1))
                            if p == 0:
                                for j in range(2):
                                    nc.vector.tensor_copy(
                                        ospill[:, hp, j, qh, :], ots[j])
                            else:
                                for j in range(2):
                                    h = 2 * hp + j
                                    osum = posb.tile([65, 512], F32, tag="osum",
                                                     name=f"osum_{hp}_{qh}_{j}")
                                    nc.vector.tensor_add(
                                        osum, ots[j], ospill[:, hp, j, qh, :])
                                    pending.append((h, qh, osum))
                            deferred.extend(pending)
                            pending = []
                            emit_tails(keep=2)
                    emit_tails()

            # ---------------- output projection ----------------
            nc.sync.dma_start(out=b_bf, in_=bout[:, :])
            for h in range(H):
                nc.sync.dma_start(out=wo12[:, h, :],
                                  in_=woT[h * 64:(h + 1) * 64, :])
            with tc.tile_pool(name="pys", bufs=2) as pys, \
                 tc.tile_pool(name="psumY", bufs=2, space="PSUM") as pY:
                for mt in range(NQ // 128):
                    yp = pY.tile([128, 1024], F32, tag="pY")
                    for h in range(H):
                        lhsT = oh_all[:, h, mt * 128:(mt + 1) * 128]
                        nc.tensor.matmul(yp[:, 0:512], lhsT, wo12[:, h, 0:512],
                                         start=(h == 0), stop=False)
                        nc.tensor.matmul(yp[:, 512:768], lhsT, wo12[:, h, 512:768],
                                         start=(h == 0), stop=False)
                    nc.tensor.matmul(yp[:, 0:512], ones_row, b_bf[:, 0:512],
                                     start=False, stop=True)
                    nc.tensor.matmul(yp[:, 512:768], ones_row, b_bf[:, 512:768],
                                     start=False, stop=True)
                    ys = pys.tile([128, D], F32, tag="ys")
                    nc.vector.tensor_copy(ys, yp[:, 0:768])
                    nc.sync.dma_start(out=y[mt * 128:(mt + 1) * 128, :], in_=ys)

    _split_multi_waits(nc)
    return nc


_prog_cache = {}


def make_in_maps(inputs):
    bf = ml_dtypes.bfloat16
    x = np.asarray(inputs["x"], dtype=np.float32)
    w_qkv = np.asarray(inputs["w_qkv"], dtype=np.float32)
    w_out = np.asarray(inputs["w_out"], dtype=np.float32)
    b_out = np.asarray(inputs["b_out"], dtype=np.float32).reshape(1, D)

    xb16 = np.ascontiguousarray(x).astype(bf)
    wqkvT = np.ascontiguousarray(w_qkv.T).astype(bf)
    woT = np.ascontiguousarray(w_out.T).astype(bf)
    b16 = b_out.astype(bf)
    pidx = np.arange(128)
    sel = (pidx[:, None] % 64 == pidx[None, :] % 64).astype(np.float32).astype(bf)

    in_maps = []
    for c in range(NCORES):
        bi, qi = c // 4, c % 4
        in_maps.append({
            "xb": xb16[bi],
            "qx": np.ascontiguousarray(xb16[bi, qi * NQ:(qi + 1) * NQ]),
            "wqkvT": wqkvT,
            "woT": woT,
            "bout": b16,
            "selin": sel,
        })
    return in_maps


def kernel(x, w_qkv, w_out, b_out, scale):
    scale = np.asarray(scale, dtype=np.float32)
    inv_scale = tuple(float(1.0 / s) for s in scale)
    nc = _prog_cache.get(inv_scale)
    if nc is None:
        nc = _build_program(inv_scale)
        _prog_cache[inv_scale] = nc

    in_maps = make_in_maps(
        {"x": x, "w_qkv": w_qkv, "w_out": w_out, "b_out": b_out})

    res = run_bass_kernel_spmd(nc, in_maps, core_ids=list(range(NCORES)))
    out = np.empty((B, N, D), dtype=np.float32)
    for c in range(NCORES):
        bi, qi = c // 4, c % 4
        out[bi, qi * NQ:(qi + 1) * NQ] = res.results[c]["y"]
    return out

